# revision 9
# baseline (speedup 1.0000x reference)
"""Multi-head attention TRN2 kernel (b=4, n=4096, e=128, h=4, d=32).

Sharding: 16 (batch, query-half) units over 8 cores; core c handles batch
c//2, query rows (c%2)*2048..+2048.  Each core computes q/k/v projections
for its batch (k,v over all 4096 keys), 4 attention heads over its 2048
query rows, and the output projection for those rows.  The host only
permutes/transposes inputs and concatenates outputs.

On-device layouts are transpose-free end to end:
  scoresT[k,q] = matmul(lhsT=kT_h, rhs=qT_h)       (K=32, head row-groups)
  expT = Exp(scoresT/sqrt(e))                       (ScalarE, the bottleneck)
  attT[hd+sum, q] = matmul(lhsT=[v_h|1], rhs=expT)  (per-head PSUM bank)
  out[q, e] = matmul(lhsT=attT_norm, rhs=W_proj)
Softmax max-subtraction is skipped (logits are ~N(0, 0.25), |logit|<6),
the value/proj biases are folded into one effective bias on the host.
"""

import os
import sys

sys.path.insert(0, "/opt/trn_rl_repo")
os.environ.setdefault("NEURON_RT_RESET_CORES", "1")

import numpy as np

E, H, D = 128, 4, 32
B, N = 4, 4096
NCORES = 8
NQ = N // 2  # per-core query rows
QB = 512  # query block
NKB = N // 128  # 32 key chunks
SCALE = float(1.0 / np.sqrt(np.float32(E)))

_CACHE = {}


def _split_multi_waits(nc):
    """This neuronxcc build accepts at most ONE sync wait per instruction;
    Tile emits up to two.  Hoist extra waits onto same-engine NoOps."""
    from concourse import mybir as mb

    for fn in nc.m.functions:
        for blk in fn.blocks:
            insts = list(blk.instructions)
            if not any(
                i.sync_info and i.sync_info.on_wait and len(i.sync_info.on_wait) > 1
                for i in insts
            ):
                continue
            new = []
            for inst in insts:
                si = inst.sync_info
                if si is not None and si.on_wait and len(si.on_wait) > 1:
                    waits = list(si.on_wait)
                    for j, w in enumerate(waits[:-1]):
                        new.append(
                            mb.InstNoOp(
                                name=f"{inst.name}-wsplit{j}",
                                engine=inst.engine,
                                ins=[],
                                outs=[],
                                sync_info=mb.SyncInfo(on_wait=[w], on_update=[]),
                            )
                        )
                    inst.sync_info = mb.SyncInfo(
                        on_wait=[waits[-1]], on_update=list(si.on_update or [])
                    )
                new.append(inst)
            blk.instructions = new


def _build(split=True):
    import concourse.bass as bass
    import concourse.tile as tile
    from concourse import mybir
    from concourse.vector_clock import ScopedClock, VectorClock

    f32 = mybir.dt.float32
    f32r = mybir.dt.float32r
    bf16 = mybir.dt.bfloat16

    class SplitDrainTileContext(tile.TileContext):
        """Final drain waits one-sem-per-instruction (walrus limit)."""

        def _drain_and_barrier(self, tick_clock, wait_clock):
            vc = tick_clock.global_clock
            n = len(vc)
            for p in range(n):
                t = vc[p]
                if t <= 0:
                    continue
                pvec = [0] * n
                pvec[p] = t
                nop_inst = self.nc.sync.nop()
                wait_clock.add_sem_waits(
                    nop_inst.ins, ScopedClock({None: VectorClock(pvec)})
                )
            self.nc.sync.drain()
            self.nc.all_engine_barrier()
            assert self.sems is not None
            popped = self.nc._tile_sem_poison_stack.pop()
            assert popped is self._sem_poison
            self.nc.clear_and_free_semaphores(list(self.sems.allocated().values()))
            self.nc.all_engine_barrier()

    nc = bass.Bass("TRN2", target_bir_lowering=False, debug=False, num_devices=NCORES)

    xT_kv = nc.dram_tensor("xT_kv", [E, N], f32, kind="ExternalInput")
    xT_q = nc.dram_tensor("xT_q", [E, NQ], f32, kind="ExternalInput")
    Wq = nc.dram_tensor("Wq", [E, E], f32, kind="ExternalInput")
    Wk = nc.dram_tensor("Wk", [E, E], f32, kind="ExternalInput")
    Wv = nc.dram_tensor("Wv", [E, E], f32, kind="ExternalInput")
    Wp = nc.dram_tensor("Wp", [E, E], f32, kind="ExternalInput")
    bq = nc.dram_tensor("bq", [E, 1], f32, kind="ExternalInput")
    bk = nc.dram_tensor("bk", [E, 1], f32, kind="ExternalInput")
    bp = nc.dram_tensor("bp", [1, E], f32, kind="ExternalInput")
    out = nc.dram_tensor("out", [NQ, E], f32, kind="ExternalOutput")

    with SplitDrainTileContext(nc) as tc:
        import contextlib

        with contextlib.ExitStack() as ctx:
            consts = ctx.enter_context(tc.tile_pool(name="consts", bufs=1))
            data = ctx.enter_context(tc.tile_pool(name="data", bufs=1))
            expool = ctx.enter_context(tc.tile_pool(name="expool", bufs=4))
            nrm = ctx.enter_context(tc.tile_pool(name="nrm", bufs=4))
            outp = ctx.enter_context(tc.tile_pool(name="outp", bufs=2))

            # ---- constants ----
            wq_s = consts.tile([E, E], f32)
            nc.gpsimd.dma_start(out=wq_s[:], in_=Wq[:])
            wk_s = consts.tile([E, E], f32)
            nc.gpsimd.dma_start(out=wk_s[:], in_=Wk[:])
            wv_s = consts.tile([E, E], f32)
            nc.gpsimd.dma_start(out=wv_s[:], in_=Wv[:])
            wp_s = consts.tile([E, E], f32)
            nc.gpsimd.dma_start(out=wp_s[:], in_=Wp[:])
            bq_s = consts.tile([E, 1], f32)
            nc.gpsimd.dma_start(out=bq_s[:], in_=bq[:])
            bk_s = consts.tile([E, 1], f32)
            nc.gpsimd.dma_start(out=bk_s[:], in_=bk[:])
            # proj bias broadcast across partitions: [1,E] -> [128,E]
            bp_s = consts.tile([E, E], f32)
            bp_bcast = bass.AP(
                tensor=bp.ap().tensor,
                offset=bp.ap().offset,
                ap=[[0, E], [1, E]],
            )
            nc.gpsimd.dma_start(out=bp_s[:], in_=bp_bcast)

            # f32r copies of the projection weights (rounded producers)
            wq_r = consts.tile([E, E], f32r)
            nc.vector.tensor_copy(wq_r[:], wq_s[:])
            wk_r = consts.tile([E, E], f32r)
            nc.vector.tensor_copy(wk_r[:], wk_s[:])
            wv_r = consts.tile([E, E], f32r)
            nc.vector.tensor_copy(wv_r[:], wv_s[:])

            # ---- x loads ----
            xq_s = data.tile([E, NQ], f32)
            xkv_s = data.tile([E, N], f32)
            for j in range(0, NQ, 1024):
                nc.gpsimd.dma_start(out=xq_s[:, j : j + 1024], in_=xT_q[:, j : j + 1024])
            for j in range(0, N, 1024):
                nc.gpsimd.dma_start(
                    out=xkv_s[:, j : j + 1024], in_=xT_kv[:, j : j + 1024]
                )
            xq_r = data.tile([E, NQ], f32r)
            xkv_r = data.tile([E, N], f32r)
            for j in range(0, NQ, 1024):
                nc.vector.tensor_copy(xq_r[:, j : j + 1024], xq_s[:, j : j + 1024])
            for j in range(0, N, 1024):
                nc.vector.tensor_copy(xkv_r[:, j : j + 1024], xkv_s[:, j : j + 1024])

            # ---- qkv projections (f32r matmuls, full-array) ----
            qT = data.tile([E, NQ], bf16)  # [ (h d), q ] with q-bias added
            kT = data.tile([E, N], bf16)  # [ (h d), k ] with k-bias added
            v1 = data.tile([E, NKB, H, D], bf16)
            ones_s = consts.tile([E, D], bf16)
            nc.vector.memset(ones_s[:], 1.0)

            pssc = ctx.enter_context(tc.tile_pool(name="pssc", bufs=2, space="PSUM"))
            psatt = ctx.enter_context(tc.tile_pool(name="psatt", bufs=4, space="PSUM"))

            def emit_qT():
                for j in range(0, NQ, QB):
                    ps = pssc.tile([E, QB], f32, tag="scps", name=f"qps{j}")
                    nc.tensor.matmul(
                        ps[:], wq_r[:], xq_r[:, j : j + QB], start=True, stop=True
                    )
                    nc.vector.tensor_scalar_add(qT[:, j : j + QB], ps[:], bq_s[:])

            def emit_kT_chunk(c):
                j = c * QB
                ps = pssc.tile([E, QB], f32, tag="scps", name=f"kps{j}")
                nc.tensor.matmul(
                    ps[:], wk_r[:], xkv_r[:, j : j + QB], start=True, stop=True
                )
                nc.vector.tensor_scalar_add(kT[:, j : j + QB], ps[:], bk_s[:])

            def emit_v_chunk(m):
                ps = pssc.tile([E, E], f32, tag="scps", name=f"vps{m}")
                nc.tensor.matmul(
                    ps[:],
                    xkv_r[:, 128 * m : 128 * m + 128],
                    wv_r[:],
                    start=True,
                    stop=True,
                )
                for h in range(H):
                    nc.vector.tensor_copy(v1[:, m, h, :], ps[:, D * h : D * h + D])

            emit_qT()
            emit_kT_chunk(0)

            # ---- attention ----
            def emit_tail(iq, att_r):
                """normalize + project + store query block iq"""
                q0 = iq * QB
                att_ps, r_ps = att_r
                rinv = nrm.tile([E, QB], f32, tag="rinv", name=f"ri{iq}")
                nc.vector.reciprocal(rinv[:], r_ps[:])
                attnT = nrm.tile([E, QB], f32, tag="attnT", name=f"attnT{iq}")
                nc.vector.tensor_mul(attnT[:], att_ps[:], rinv[:])
                pp = pssc.tile([E, QB], f32, tag="scps", name=f"pp{iq}")
                for m in range(QB // 128):
                    nc.tensor.matmul(
                        pp[:, 128 * m : 128 * m + 128],
                        attnT[:, 128 * m : 128 * m + 128],
                        wp_s[:],
                        start=(m == 0),
                        stop=(m == QB // 128 - 1),
                        skip_group_check=True,
                    )
                ob = outp.tile([E, QB], f32, tag="ob", name=f"ob{iq}")
                bp_rep = bass.AP(
                    tensor=bp_s[:].tensor,
                    offset=bp_s[:].offset,
                    ap=[list(bp_s[:].ap[0]), [0, QB // 128], [1, E]],
                )
                ob_v = ob[:].rearrange("p (m e) -> p m e", e=E)
                pp_v = pp[:].rearrange("p (m e) -> p m e", e=E)
                nc.vector.tensor_add(ob_v, pp_v, bp_rep)
                for m in range(QB // 128):
                    nc.gpsimd.dma_start(
                        out=out[q0 + 128 * m : q0 + 128 * m + 128, :],
                        in_=ob[:, 128 * m : 128 * m + 128],
                    )

            prev_ar = None
            prev_iq = None
            for iq in range(NQ // QB):
                q0 = iq * QB
                att_ps = psatt.tile([E, QB], f32, tag="attps", name=f"attp{iq}")
                r_ps = psatt.tile([E, QB], f32, tag="attps", name=f"rp{iq}")
                for k in range(NKB):
                    k0 = 128 * k
                    if iq == 0:
                        # stream the rest of the kv projections under the
                        # first query block's attention
                        if k % 4 == 0 and (k // 4 + 1) < N // QB:
                            emit_kT_chunk(k // 4 + 1)
                        emit_v_chunk(k)
                    scs = [
                        pssc.tile([E, 2 * QB], f32, tag="scps", name=f"sc{iq}_{k}_{p}")
                        for p in range(2)
                    ]
                    # 4-way row-group packed score matmuls
                    for h in range(H):
                        nc.tensor.matmul(
                            scs[h // 2][:, QB * (h % 2) : QB * (h % 2) + QB],
                            kT[D * h : D * h + D, k0 : k0 + 128],
                            qT[D * h : D * h + D, q0 : q0 + QB],
                            start=True,
                            stop=True,
                            tile_position=(D * h, 0),
                        )
                    exs = []
                    for p in range(2):
                        ex = expool.tile(
                            [E, 2 * QB], bf16, tag="ex", name=f"ex{iq}_{k}_{p}"
                        )
                        nc.scalar.activation(
                            out=ex[:],
                            in_=scs[p][:],
                            func=mybir.ActivationFunctionType.Exp,
                            scale=SCALE,
                        )
                        exs.append(ex)
                    # 4-way column-group packed att and rowsum accumulation
                    for h in range(H):
                        nc.tensor.matmul(
                            att_ps[D * h : D * h + D, :],
                            v1[:, k, h, :],
                            exs[h // 2][:, QB * (h % 2) : QB * (h % 2) + QB],
                            start=(k == 0),
                            stop=(k == NKB - 1),
                            tile_position=(0, D * h),
                            skip_group_check=True,
                        )
                    for h in range(H):
                        nc.tensor.matmul(
                            r_ps[D * h : D * h + D, :],
                            ones_s[:],
                            exs[h // 2][:, QB * (h % 2) : QB * (h % 2) + QB],
                            start=(k == 0),
                            stop=(k == NKB - 1),
                            tile_position=(0, D * h),
                            skip_group_check=True,
                        )
                    if k == 6 and prev_ar is not None:
                        emit_tail(prev_iq, prev_ar)
                prev_ar, prev_iq = (att_ps, r_ps), iq
            emit_tail(prev_iq, prev_ar)

    if split:
        _split_multi_waits(nc)
    return nc


def _prep_host(x, W_qkv, b_qkv, W_proj, b_proj):
    j = np.arange(E)
    h, d = j // D, j % D
    cq = h * (3 * D) + d * 3 + 0
    ck = cq + 1
    cv = cq + 2
    Wq = np.ascontiguousarray(W_qkv[:, cq], np.float32)
    Wk = np.ascontiguousarray(W_qkv[:, ck], np.float32)
    Wv = np.ascontiguousarray(W_qkv[:, cv], np.float32)
    bq = np.ascontiguousarray(b_qkv[cq].reshape(E, 1), np.float32)
    bk = np.ascontiguousarray(b_qkv[ck].reshape(E, 1), np.float32)
    bv = b_qkv[cv].astype(np.float32)
    bp = (bv @ W_proj + b_proj).astype(np.float32).reshape(1, E)
    Wp = np.ascontiguousarray(W_proj, np.float32)
    in_maps = []
    for c in range(NCORES):
        b, half = c // 2, c % 2
        xT_kv = np.ascontiguousarray(x[b].T, np.float32)
        xT_q = np.ascontiguousarray(x[b, half * NQ : (half + 1) * NQ].T, np.float32)
        in_maps.append(
            {
                "xT_kv": xT_kv,
                "xT_q": xT_q,
                "Wq": Wq,
                "Wk": Wk,
                "Wv": Wv,
                "Wp": Wp,
                "bq": bq,
                "bk": bk,
                "bp": bp,
            }
        )
    return in_maps


def kernel(x, W_qkv, b_qkv, W_proj, b_proj, _trace=False):
    x = np.asarray(x, np.float32)
    W_qkv = np.asarray(W_qkv, np.float32)
    b_qkv = np.asarray(b_qkv, np.float32)
    W_proj = np.asarray(W_proj, np.float32)
    b_proj = np.asarray(b_proj, np.float32)

    from concourse.bass_utils import run_bass_kernel_spmd

    if "nc" not in _CACHE:
        _CACHE["nc"] = _build()
    nc = _CACHE["nc"]

    in_maps = _prep_host(x, W_qkv, b_qkv, W_proj, b_proj)
    res = run_bass_kernel_spmd(
        nc, in_maps, core_ids=list(range(NCORES)), trace=_trace
    )
    out = np.empty((B, N, E), np.float32)
    for c in range(NCORES):
        b, half = c // 2, c % 2
        out[b, half * NQ : (half + 1) * NQ] = res.results[c]["out"]
    if _trace:
        _CACHE["last_result"] = res
    return out


# revision 10
# speedup vs baseline: 1.2138x; 1.2138x over previous
"""Multi-head attention TRN2 kernel (b=4, n=4096, e=128, h=4, d=32).

Sharding: 16 (batch, query-half) units over 8 cores; core c handles batch
c//2, query rows (c%2)*2048..+2048.  Each core computes q/k/v projections
for its batch (k,v over all 4096 keys), 4 attention heads over its 2048
query rows, and the output projection for those rows.  The host only
permutes/transposes inputs and concatenates outputs.

On-device layouts are transpose-free end to end:
  scoresT[k,q] = matmul(lhsT=kT_h, rhs=qT_h)       (K=32, head row-groups)
  expT = Exp(scoresT/sqrt(e))                       (ScalarE, the bottleneck)
  attT[hd+sum, q] = matmul(lhsT=[v_h|1], rhs=expT)  (per-head PSUM bank)
  out[q, e] = matmul(lhsT=attT_norm, rhs=W_proj)
Softmax max-subtraction is skipped (logits are ~N(0, 0.25), |logit|<6),
the value/proj biases are folded into one effective bias on the host.
"""

import os
import sys

sys.path.insert(0, "/opt/trn_rl_repo")
os.environ.setdefault("NEURON_RT_RESET_CORES", "1")

import numpy as np

E, H, D = 128, 4, 32
B, N = 4, 4096
NCORES = 8
NQ = N // 2  # per-core query rows
QB = 512  # query block
NKB = N // 128  # 32 key chunks
SCALE = float(1.0 / np.sqrt(np.float32(E)))

_CACHE = {}


def _split_multi_waits(nc):
    """This neuronxcc build accepts at most ONE sync wait per instruction;
    Tile emits up to two.  Hoist extra waits onto same-engine NoOps."""
    from concourse import mybir as mb

    for fn in nc.m.functions:
        for blk in fn.blocks:
            insts = list(blk.instructions)
            if not any(
                i.sync_info and i.sync_info.on_wait and len(i.sync_info.on_wait) > 1
                for i in insts
            ):
                continue
            new = []
            for inst in insts:
                si = inst.sync_info
                if si is not None and si.on_wait and len(si.on_wait) > 1:
                    waits = list(si.on_wait)
                    for j, w in enumerate(waits[:-1]):
                        new.append(
                            mb.InstNoOp(
                                name=f"{inst.name}-wsplit{j}",
                                engine=inst.engine,
                                ins=[],
                                outs=[],
                                sync_info=mb.SyncInfo(on_wait=[w], on_update=[]),
                            )
                        )
                    inst.sync_info = mb.SyncInfo(
                        on_wait=[waits[-1]], on_update=list(si.on_update or [])
                    )
                new.append(inst)
            blk.instructions = new


def _build(split=True):
    import concourse.bass as bass
    import concourse.tile as tile
    from concourse import mybir
    from concourse.vector_clock import ScopedClock, VectorClock

    f32 = mybir.dt.float32
    f32r = mybir.dt.float32r
    bf16 = mybir.dt.bfloat16

    class SplitDrainTileContext(tile.TileContext):
        """Final drain waits one-sem-per-instruction (walrus limit)."""

        def _drain_and_barrier(self, tick_clock, wait_clock):
            vc = tick_clock.global_clock
            n = len(vc)
            for p in range(n):
                t = vc[p]
                if t <= 0:
                    continue
                pvec = [0] * n
                pvec[p] = t
                nop_inst = self.nc.sync.nop()
                wait_clock.add_sem_waits(
                    nop_inst.ins, ScopedClock({None: VectorClock(pvec)})
                )
            self.nc.sync.drain()
            self.nc.all_engine_barrier()
            assert self.sems is not None
            popped = self.nc._tile_sem_poison_stack.pop()
            assert popped is self._sem_poison
            self.nc.clear_and_free_semaphores(list(self.sems.allocated().values()))
            self.nc.all_engine_barrier()

    nc = bass.Bass("TRN2", target_bir_lowering=False, debug=False, num_devices=NCORES)

    xT_kv = nc.dram_tensor("xT_kv", [E, N], f32, kind="ExternalInput")
    xT_q = nc.dram_tensor("xT_q", [E, NQ], f32, kind="ExternalInput")
    Wq = nc.dram_tensor("Wq", [E, E], f32, kind="ExternalInput")
    Wk = nc.dram_tensor("Wk", [E, E], f32, kind="ExternalInput")
    Wv = nc.dram_tensor("Wv", [E, E], f32, kind="ExternalInput")
    Wp = nc.dram_tensor("Wp", [E, E], f32, kind="ExternalInput")
    bq = nc.dram_tensor("bq", [E, 1], f32, kind="ExternalInput")
    bk = nc.dram_tensor("bk", [E, 1], f32, kind="ExternalInput")
    bp = nc.dram_tensor("bp", [1, E], f32, kind="ExternalInput")
    out = nc.dram_tensor("out", [NQ, E], f32, kind="ExternalOutput")

    with SplitDrainTileContext(nc) as tc:
        import contextlib

        with contextlib.ExitStack() as ctx:
            consts = ctx.enter_context(tc.tile_pool(name="consts", bufs=1))
            data = ctx.enter_context(tc.tile_pool(name="data", bufs=1))
            expool = ctx.enter_context(tc.tile_pool(name="expool", bufs=4))
            nrm = ctx.enter_context(tc.tile_pool(name="nrm", bufs=4))
            outp = ctx.enter_context(tc.tile_pool(name="outp", bufs=2))

            # ---- constants ----
            wq_s = consts.tile([E, E], f32)
            nc.gpsimd.dma_start(out=wq_s[:], in_=Wq[:])
            wk_s = consts.tile([E, E], f32)
            nc.gpsimd.dma_start(out=wk_s[:], in_=Wk[:])
            wv_s = consts.tile([E, E], f32)
            nc.gpsimd.dma_start(out=wv_s[:], in_=Wv[:])
            wp_s = consts.tile([E, E], f32)
            nc.gpsimd.dma_start(out=wp_s[:], in_=Wp[:])
            bq_s = consts.tile([E, 1], f32)
            nc.gpsimd.dma_start(out=bq_s[:], in_=bq[:])
            bk_s = consts.tile([E, 1], f32)
            nc.gpsimd.dma_start(out=bk_s[:], in_=bk[:])
            # proj bias broadcast across partitions: [1,E] -> [128,E]
            bp_s = consts.tile([E, E], f32)
            bp_bcast = bass.AP(
                tensor=bp.ap().tensor,
                offset=bp.ap().offset,
                ap=[[0, E], [1, E]],
            )
            nc.gpsimd.dma_start(out=bp_s[:], in_=bp_bcast)

            # f32r copies of the projection weights (rounded producers)
            wq_r = consts.tile([E, E], f32r)
            nc.vector.tensor_copy(wq_r[:], wq_s[:])
            wk_r = consts.tile([E, E], f32r)
            nc.vector.tensor_copy(wk_r[:], wk_s[:])
            wv_r = consts.tile([E, E], f32r)
            nc.vector.tensor_copy(wv_r[:], wv_s[:])

            # ---- x loads ----
            xq_s = data.tile([E, NQ], f32)
            xkv_s = data.tile([E, N], f32)
            for j in range(0, NQ, 1024):
                nc.gpsimd.dma_start(out=xq_s[:, j : j + 1024], in_=xT_q[:, j : j + 1024])
            for j in range(0, N, 1024):
                nc.gpsimd.dma_start(
                    out=xkv_s[:, j : j + 1024], in_=xT_kv[:, j : j + 1024]
                )
            xq_r = data.tile([E, NQ], f32r)
            xkv_r = data.tile([E, N], f32r)
            for j in range(0, NQ, 1024):
                nc.vector.tensor_copy(xq_r[:, j : j + 1024], xq_s[:, j : j + 1024])
            for j in range(0, N, 1024):
                nc.vector.tensor_copy(xkv_r[:, j : j + 1024], xkv_s[:, j : j + 1024])

            # ---- qkv projections (f32r matmuls, full-array) ----
            qT = data.tile([E, NQ], bf16)  # [ (h d), q ] with q-bias added
            kT = data.tile([E, N], bf16)  # [ (h d), k ] with k-bias added
            v1 = data.tile([E, NKB, H, D], bf16)
            ones_s = consts.tile([E, D], bf16)
            nc.vector.memset(ones_s[:], 1.0)

            pssc = ctx.enter_context(tc.tile_pool(name="pssc", bufs=3, space="PSUM"))
            psatt = ctx.enter_context(tc.tile_pool(name="psatt", bufs=2, space="PSUM"))

            def emit_qT():
                for j in range(0, NQ, QB):
                    ps = pssc.tile([E, QB], f32, tag="scps", name=f"qps{j}")
                    nc.tensor.matmul(
                        ps[:], wq_r[:], xq_r[:, j : j + QB], start=True, stop=True
                    )
                    nc.vector.tensor_scalar_add(qT[:, j : j + QB], ps[:], bq_s[:])

            def emit_kT_chunk(c):
                j = c * QB
                ps = pssc.tile([E, QB], f32, tag="scps", name=f"kps{j}")
                nc.tensor.matmul(
                    ps[:], wk_r[:], xkv_r[:, j : j + QB], start=True, stop=True
                )
                nc.vector.tensor_scalar_add(kT[:, j : j + QB], ps[:], bk_s[:])

            def emit_v_chunk(m):
                ps = pssc.tile([E, E], f32, tag="scps", name=f"vps{m}")
                nc.tensor.matmul(
                    ps[:],
                    xkv_r[:, 128 * m : 128 * m + 128],
                    wv_r[:],
                    start=True,
                    stop=True,
                )
                for h in range(H):
                    nc.vector.tensor_copy(v1[:, m, h, :], ps[:, D * h : D * h + D])

            emit_qT()
            emit_kT_chunk(0)

            # ---- attention ----
            def emit_tail(iq, att_r):
                """normalize + project + store query block iq"""
                q0 = iq * QB
                att_ps, r_ps = att_r
                rinv = nrm.tile([E, QB], f32, tag="rinv", name=f"ri{iq}")
                nc.vector.reciprocal(rinv[:], r_ps[:])
                attnT = nrm.tile([E, QB], f32, tag="attnT", name=f"attnT{iq}")
                nc.vector.tensor_mul(attnT[:], att_ps[:], rinv[:])
                pp = pssc.tile([E, QB], f32, tag="scps", name=f"pp{iq}")
                for m in range(QB // 128):
                    nc.tensor.matmul(
                        pp[:, 128 * m : 128 * m + 128],
                        attnT[:, 128 * m : 128 * m + 128],
                        wp_s[:],
                        start=(m == 0),
                        stop=(m == QB // 128 - 1),
                        skip_group_check=True,
                    )
                ob = outp.tile([E, QB], f32, tag="ob", name=f"ob{iq}")
                bp_rep = bass.AP(
                    tensor=bp_s[:].tensor,
                    offset=bp_s[:].offset,
                    ap=[list(bp_s[:].ap[0]), [0, QB // 128], [1, E]],
                )
                ob_v = ob[:].rearrange("p (m e) -> p m e", e=E)
                pp_v = pp[:].rearrange("p (m e) -> p m e", e=E)
                nc.vector.tensor_add(ob_v, pp_v, bp_rep)
                for m in range(QB // 128):
                    nc.gpsimd.dma_start(
                        out=out[q0 + 128 * m : q0 + 128 * m + 128, :],
                        in_=ob[:, 128 * m : 128 * m + 128],
                    )

            for iq in range(NQ // QB):
                q0 = iq * QB
                att_ps = psatt.tile([E, QB], f32, tag="attps", name=f"attp{iq}")
                r_ps = psatt.tile([E, QB], f32, tag="attps", name=f"rp{iq}")
                for k in range(NKB):
                    k0 = 128 * k
                    if iq == 0:
                        # stream the rest of the kv projections under the
                        # first query block's attention
                        if k % 4 == 0 and (k // 4 + 1) < N // QB:
                            emit_kT_chunk(k // 4 + 1)
                        emit_v_chunk(k)
                    scs = [
                        pssc.tile([E, 2 * QB], f32, tag="scps", name=f"sc{iq}_{k}_{p}")
                        for p in range(2)
                    ]
                    # 4-way row-group packed score matmuls
                    for h in range(H):
                        nc.tensor.matmul(
                            scs[h // 2][:, QB * (h % 2) : QB * (h % 2) + QB],
                            kT[D * h : D * h + D, k0 : k0 + 128],
                            qT[D * h : D * h + D, q0 : q0 + QB],
                            start=True,
                            stop=True,
                            tile_position=(D * h, 0),
                        )
                    exs = []
                    for p in range(2):
                        ex = expool.tile(
                            [E, 2 * QB], bf16, tag="ex", name=f"ex{iq}_{k}_{p}"
                        )
                        nc.scalar.activation(
                            out=ex[:],
                            in_=scs[p][:],
                            func=mybir.ActivationFunctionType.Exp,
                            scale=SCALE,
                        )
                        exs.append(ex)
                    # 4-way column-group packed att and rowsum accumulation
                    for h in range(H):
                        nc.tensor.matmul(
                            att_ps[D * h : D * h + D, :],
                            v1[:, k, h, :],
                            exs[h // 2][:, QB * (h % 2) : QB * (h % 2) + QB],
                            start=(k == 0),
                            stop=(k == NKB - 1),
                            tile_position=(0, D * h),
                            skip_group_check=True,
                        )
                    for h in range(H):
                        nc.tensor.matmul(
                            r_ps[D * h : D * h + D, :],
                            ones_s[:],
                            exs[h // 2][:, QB * (h % 2) : QB * (h % 2) + QB],
                            start=(k == 0),
                            stop=(k == NKB - 1),
                            tile_position=(0, D * h),
                            skip_group_check=True,
                        )
                emit_tail(iq, (att_ps, r_ps))

    if split:
        _split_multi_waits(nc)
    return nc


def _prep_host(x, W_qkv, b_qkv, W_proj, b_proj):
    j = np.arange(E)
    h, d = j // D, j % D
    cq = h * (3 * D) + d * 3 + 0
    ck = cq + 1
    cv = cq + 2
    Wq = np.ascontiguousarray(W_qkv[:, cq], np.float32)
    Wk = np.ascontiguousarray(W_qkv[:, ck], np.float32)
    Wv = np.ascontiguousarray(W_qkv[:, cv], np.float32)
    bq = np.ascontiguousarray(b_qkv[cq].reshape(E, 1), np.float32)
    bk = np.ascontiguousarray(b_qkv[ck].reshape(E, 1), np.float32)
    bv = b_qkv[cv].astype(np.float32)
    bp = (bv @ W_proj + b_proj).astype(np.float32).reshape(1, E)
    Wp = np.ascontiguousarray(W_proj, np.float32)
    in_maps = []
    for c in range(NCORES):
        b, half = c // 2, c % 2
        xT_kv = np.ascontiguousarray(x[b].T, np.float32)
        xT_q = np.ascontiguousarray(x[b, half * NQ : (half + 1) * NQ].T, np.float32)
        in_maps.append(
            {
                "xT_kv": xT_kv,
                "xT_q": xT_q,
                "Wq": Wq,
                "Wk": Wk,
                "Wv": Wv,
                "Wp": Wp,
                "bq": bq,
                "bk": bk,
                "bp": bp,
            }
        )
    return in_maps


def kernel(x, W_qkv, b_qkv, W_proj, b_proj, _trace=False):
    x = np.asarray(x, np.float32)
    W_qkv = np.asarray(W_qkv, np.float32)
    b_qkv = np.asarray(b_qkv, np.float32)
    W_proj = np.asarray(W_proj, np.float32)
    b_proj = np.asarray(b_proj, np.float32)

    from concourse.bass_utils import run_bass_kernel_spmd

    if "nc" not in _CACHE:
        _CACHE["nc"] = _build()
    nc = _CACHE["nc"]

    in_maps = _prep_host(x, W_qkv, b_qkv, W_proj, b_proj)
    res = run_bass_kernel_spmd(
        nc, in_maps, core_ids=list(range(NCORES)), trace=_trace
    )
    out = np.empty((B, N, E), np.float32)
    for c in range(NCORES):
        b, half = c // 2, c % 2
        out[b, half * NQ : (half + 1) * NQ] = res.results[c]["out"]
    if _trace:
        _CACHE["last_result"] = res
    return out


# revision 11
# speedup vs baseline: 1.5737x; 1.2965x over previous
"""Multi-head attention TRN2 kernel (b=4, n=4096, e=128, h=4, d=32).

Sharding: 16 (batch, query-half) units over 8 cores; core c handles batch
c//2, query rows (c%2)*2048..+2048.  Each core computes q/k/v projections
for its batch (k,v over all 4096 keys), 4 attention heads over its 2048
query rows, and the output projection for those rows.  The host only
permutes/transposes inputs and concatenates outputs.

On-device layouts are transpose-free end to end:
  scoresT[k,q] = matmul(lhsT=kT_h, rhs=qT_h)       (K=32, head row-groups)
  expT = Exp(scoresT/sqrt(e))                       (ScalarE, the bottleneck)
  attT[hd+sum, q] = matmul(lhsT=[v_h|1], rhs=expT)  (per-head PSUM bank)
  out[q, e] = matmul(lhsT=attT_norm, rhs=W_proj)
Softmax max-subtraction is skipped (logits are ~N(0, 0.25), |logit|<6),
the value/proj biases are folded into one effective bias on the host.
"""

import os
import sys

sys.path.insert(0, "/opt/trn_rl_repo")
os.environ.setdefault("NEURON_RT_RESET_CORES", "1")

import numpy as np

E, H, D = 128, 4, 32
B, N = 4, 4096
NCORES = 8
NQ = N // 2  # per-core query rows
QB = 512  # query block
NKB = N // 128  # 32 key chunks
SCALE = float(1.0 / np.sqrt(np.float32(E)))

_CACHE = {}


def _split_multi_waits(nc):
    """This neuronxcc build accepts at most ONE sync wait per instruction;
    Tile emits up to two.  Hoist extra waits onto same-engine NoOps."""
    from concourse import mybir as mb

    for fn in nc.m.functions:
        for blk in fn.blocks:
            insts = list(blk.instructions)
            if not any(
                i.sync_info and i.sync_info.on_wait and len(i.sync_info.on_wait) > 1
                for i in insts
            ):
                continue
            new = []
            for inst in insts:
                si = inst.sync_info
                if si is not None and si.on_wait and len(si.on_wait) > 1:
                    waits = list(si.on_wait)
                    for j, w in enumerate(waits[:-1]):
                        new.append(
                            mb.InstNoOp(
                                name=f"{inst.name}-wsplit{j}",
                                engine=inst.engine,
                                ins=[],
                                outs=[],
                                sync_info=mb.SyncInfo(on_wait=[w], on_update=[]),
                            )
                        )
                    inst.sync_info = mb.SyncInfo(
                        on_wait=[waits[-1]], on_update=list(si.on_update or [])
                    )
                new.append(inst)
            blk.instructions = new


def _build(split=True):
    import concourse.bass as bass
    import concourse.tile as tile
    from concourse import mybir
    from concourse.vector_clock import ScopedClock, VectorClock

    f32 = mybir.dt.float32
    f32r = mybir.dt.float32r
    bf16 = mybir.dt.bfloat16

    class SplitDrainTileContext(tile.TileContext):
        """Final drain waits one-sem-per-instruction (walrus limit)."""

        def _drain_and_barrier(self, tick_clock, wait_clock):
            vc = tick_clock.global_clock
            n = len(vc)
            for p in range(n):
                t = vc[p]
                if t <= 0:
                    continue
                pvec = [0] * n
                pvec[p] = t
                nop_inst = self.nc.sync.nop()
                wait_clock.add_sem_waits(
                    nop_inst.ins, ScopedClock({None: VectorClock(pvec)})
                )
            self.nc.sync.drain()
            self.nc.all_engine_barrier()
            assert self.sems is not None
            popped = self.nc._tile_sem_poison_stack.pop()
            assert popped is self._sem_poison
            self.nc.clear_and_free_semaphores(list(self.sems.allocated().values()))
            self.nc.all_engine_barrier()

    nc = bass.Bass("TRN2", target_bir_lowering=False, debug=False, num_devices=NCORES)

    xT_kv = nc.dram_tensor("xT_kv", [E, N], f32, kind="ExternalInput")
    xT_q = nc.dram_tensor("xT_q", [E, NQ], f32, kind="ExternalInput")
    Wq = nc.dram_tensor("Wq", [E, E], f32, kind="ExternalInput")
    Wk = nc.dram_tensor("Wk", [E, E], f32, kind="ExternalInput")
    Wv = nc.dram_tensor("Wv", [E, E], f32, kind="ExternalInput")
    Wp = nc.dram_tensor("Wp", [E, E], f32, kind="ExternalInput")
    bq = nc.dram_tensor("bq", [E, 1], f32, kind="ExternalInput")
    bk = nc.dram_tensor("bk", [E, 1], f32, kind="ExternalInput")
    bp = nc.dram_tensor("bp", [1, E], f32, kind="ExternalInput")
    out = nc.dram_tensor("out", [NQ, E], f32, kind="ExternalOutput")

    with SplitDrainTileContext(nc) as tc:
        import contextlib

        with contextlib.ExitStack() as ctx:
            consts = ctx.enter_context(tc.tile_pool(name="consts", bufs=1))
            data = ctx.enter_context(tc.tile_pool(name="data", bufs=1))
            expool = ctx.enter_context(tc.tile_pool(name="expool", bufs=4))
            nrm = ctx.enter_context(tc.tile_pool(name="nrm", bufs=4))
            outp = ctx.enter_context(tc.tile_pool(name="outp", bufs=2))

            # ---- constants ----
            wq_s = consts.tile([E, E], f32)
            nc.gpsimd.dma_start(out=wq_s[:], in_=Wq[:])
            wk_s = consts.tile([E, E], f32)
            nc.gpsimd.dma_start(out=wk_s[:], in_=Wk[:])
            wv_s = consts.tile([E, E], f32)
            nc.gpsimd.dma_start(out=wv_s[:], in_=Wv[:])
            wp_s = consts.tile([E, E], f32)
            nc.gpsimd.dma_start(out=wp_s[:], in_=Wp[:])
            bq_s = consts.tile([E, 1], f32)
            nc.gpsimd.dma_start(out=bq_s[:], in_=bq[:])
            bk_s = consts.tile([E, 1], f32)
            nc.gpsimd.dma_start(out=bk_s[:], in_=bk[:])
            # proj bias broadcast across partitions: [1,E] -> [128,E]
            bp_s = consts.tile([E, E], f32)
            bp_bcast = bass.AP(
                tensor=bp.ap().tensor,
                offset=bp.ap().offset,
                ap=[[0, E], [1, E]],
            )
            nc.gpsimd.dma_start(out=bp_s[:], in_=bp_bcast)

            # ---- x loads ----
            xq_s = data.tile([E, NQ], f32)
            xkv_s = data.tile([E, N], f32)
            for j in range(0, NQ, QB):
                nc.gpsimd.dma_start(out=xq_s[:, j : j + QB], in_=xT_q[:, j : j + QB])
            for j in range(0, N, 1024):
                nc.gpsimd.dma_start(
                    out=xkv_s[:, j : j + 1024], in_=xT_kv[:, j : j + 1024]
                )

            # ---- qkv projections (f32r matmuls, full-array) ----
            qT = data.tile([E, NQ], bf16)  # [ (h d), q ] with q-bias added
            kT = data.tile([E, N], bf16)  # [ (h d), k ] with k-bias added
            v1 = data.tile([E, NKB, H, D], bf16)
            ones_s = consts.tile([E, D], bf16)
            nc.vector.memset(ones_s[:], 1.0)

            pssc = ctx.enter_context(tc.tile_pool(name="pssc", bufs=3, space="PSUM"))
            psatt = ctx.enter_context(tc.tile_pool(name="psatt", bufs=2, space="PSUM"))

            def emit_qT():
                for j in range(0, NQ, QB):
                    ps = pssc.tile([E, QB], f32, tag="scps", name=f"qps{j}")
                    nc.tensor.matmul(
                        ps[:], wq_s[:], xq_s[:, j : j + QB], start=True, stop=True
                    )
                    nc.vector.tensor_scalar_add(qT[:, j : j + QB], ps[:], bq_s[:])

            def emit_kT_chunk(c):
                j = c * QB
                ps = pssc.tile([E, QB], f32, tag="scps", name=f"kps{j}")
                nc.tensor.matmul(
                    ps[:], wk_s[:], xkv_s[:, j : j + QB], start=True, stop=True
                )
                nc.vector.tensor_scalar_add(kT[:, j : j + QB], ps[:], bk_s[:])

            def emit_v_chunk(m):
                ps = pssc.tile([E, E], f32, tag="scps", name=f"vps{m}")
                nc.tensor.matmul(
                    ps[:],
                    xkv_s[:, 128 * m : 128 * m + 128],
                    wv_s[:],
                    start=True,
                    stop=True,
                )
                for h in range(H):
                    nc.vector.tensor_copy(v1[:, m, h, :], ps[:, D * h : D * h + D])

            emit_qT()
            emit_kT_chunk(0)

            # ---- attention ----
            def emit_tail(iq, att_r):
                """normalize + project + store query block iq"""
                q0 = iq * QB
                att_ps, r_ps = att_r
                rinv = nrm.tile([E, QB], f32, tag="rinv", name=f"ri{iq}")
                nc.vector.reciprocal(rinv[:], r_ps[:])
                attnT = nrm.tile([E, QB], f32, tag="attnT", name=f"attnT{iq}")
                nc.vector.tensor_mul(attnT[:], att_ps[:], rinv[:])
                pp = pssc.tile([E, QB], f32, tag="scps", name=f"pp{iq}")
                for m in range(QB // 128):
                    nc.tensor.matmul(
                        pp[:, 128 * m : 128 * m + 128],
                        attnT[:, 128 * m : 128 * m + 128],
                        wp_s[:],
                        start=(m == 0),
                        stop=(m == QB // 128 - 1),
                        skip_group_check=True,
                    )
                ob = outp.tile([E, QB], f32, tag="ob", name=f"ob{iq}")
                bp_rep = bass.AP(
                    tensor=bp_s[:].tensor,
                    offset=bp_s[:].offset,
                    ap=[list(bp_s[:].ap[0]), [0, QB // 128], [1, E]],
                )
                ob_v = ob[:].rearrange("p (m e) -> p m e", e=E)
                pp_v = pp[:].rearrange("p (m e) -> p m e", e=E)
                nc.vector.tensor_add(ob_v, pp_v, bp_rep)
                for m in range(QB // 128):
                    nc.gpsimd.dma_start(
                        out=out[q0 + 128 * m : q0 + 128 * m + 128, :],
                        in_=ob[:, 128 * m : 128 * m + 128],
                    )

            NSC = NQ // QB * NKB * 2  # 256 half-iterations (qb, k, pair)

            def sc_tile(jj):
                return pssc.tile([E, 2 * QB], f32, tag="scps", name=f"sc{jj}")

            def emit_sc(jj, sc):
                qb, k, p = jj // (2 * NKB), (jj % (2 * NKB)) // 2, jj % 2
                q0, k0 = qb * QB, 128 * k
                for hh in range(2):
                    h = 2 * p + hh
                    nc.tensor.matmul(
                        sc[:, QB * hh : QB * hh + QB],
                        kT[D * h : D * h + D, k0 : k0 + 128],
                        qT[D * h : D * h + D, q0 : q0 + QB],
                        start=True,
                        stop=True,
                        tile_position=(D * h, 0),
                    )

            acc = {}
            scs = {0: sc_tile(0)}
            emit_sc(0, scs[0])
            for jj in range(NSC):
                qb, k, p = jj // (2 * NKB), (jj % (2 * NKB)) // 2, jj % 2
                if qb == 0 and p == 0:
                    # stream the kv projections under the first query block
                    if k % 4 == 0 and (k // 4 + 1) < N // QB:
                        emit_kT_chunk(k // 4 + 1)
                    emit_v_chunk(k)
                if k == 0 and p == 0:
                    att_ps = psatt.tile([E, QB], f32, tag="attps", name=f"attp{qb}")
                    r_ps = psatt.tile([E, QB], f32, tag="attps", name=f"rp{qb}")
                    acc[qb] = (att_ps, r_ps)
                if jj + 1 < NSC:
                    scs[jj + 1] = sc_tile(jj + 1)
                    emit_sc(jj + 1, scs[jj + 1])
                sc = scs.pop(jj)
                ex = expool.tile([E, 2 * QB], bf16, tag="ex", name=f"ex{jj}")
                nc.scalar.activation(
                    out=ex[:],
                    in_=sc[:],
                    func=mybir.ActivationFunctionType.Exp,
                    scale=SCALE,
                )
                att_ps, r_ps = acc[qb]
                for hh in range(2):
                    h = 2 * p + hh
                    nc.tensor.matmul(
                        att_ps[D * h : D * h + D, :],
                        v1[:, k, h, :],
                        ex[:, QB * hh : QB * hh + QB],
                        start=(k == 0),
                        stop=(k == NKB - 1),
                        tile_position=(0, D * h),
                        skip_group_check=True,
                    )
                    nc.tensor.matmul(
                        r_ps[D * h : D * h + D, :],
                        ones_s[:],
                        ex[:, QB * hh : QB * hh + QB],
                        start=(k == 0),
                        stop=(k == NKB - 1),
                        tile_position=(0, D * h),
                        skip_group_check=True,
                    )
                if k == NKB - 1 and p == 1:
                    emit_tail(qb, acc.pop(qb))

    if split:
        _split_multi_waits(nc)
    return nc


def _prep_host(x, W_qkv, b_qkv, W_proj, b_proj):
    j = np.arange(E)
    h, d = j // D, j % D
    cq = h * (3 * D) + d * 3 + 0
    ck = cq + 1
    cv = cq + 2
    Wq = np.ascontiguousarray(W_qkv[:, cq], np.float32)
    Wk = np.ascontiguousarray(W_qkv[:, ck], np.float32)
    Wv = np.ascontiguousarray(W_qkv[:, cv], np.float32)
    bq = np.ascontiguousarray(b_qkv[cq].reshape(E, 1), np.float32)
    bk = np.ascontiguousarray(b_qkv[ck].reshape(E, 1), np.float32)
    bv = b_qkv[cv].astype(np.float32)
    bp = (bv @ W_proj + b_proj).astype(np.float32).reshape(1, E)
    Wp = np.ascontiguousarray(W_proj, np.float32)
    in_maps = []
    for c in range(NCORES):
        b, half = c // 2, c % 2
        xT_kv = np.ascontiguousarray(x[b].T, np.float32)
        xT_q = np.ascontiguousarray(x[b, half * NQ : (half + 1) * NQ].T, np.float32)
        in_maps.append(
            {
                "xT_kv": xT_kv,
                "xT_q": xT_q,
                "Wq": Wq,
                "Wk": Wk,
                "Wv": Wv,
                "Wp": Wp,
                "bq": bq,
                "bk": bk,
                "bp": bp,
            }
        )
    return in_maps


def kernel(x, W_qkv, b_qkv, W_proj, b_proj, _trace=False):
    x = np.asarray(x, np.float32)
    W_qkv = np.asarray(W_qkv, np.float32)
    b_qkv = np.asarray(b_qkv, np.float32)
    W_proj = np.asarray(W_proj, np.float32)
    b_proj = np.asarray(b_proj, np.float32)

    from concourse.bass_utils import run_bass_kernel_spmd

    if "nc" not in _CACHE:
        _CACHE["nc"] = _build()
    nc = _CACHE["nc"]

    in_maps = _prep_host(x, W_qkv, b_qkv, W_proj, b_proj)
    res = run_bass_kernel_spmd(
        nc, in_maps, core_ids=list(range(NCORES)), trace=_trace
    )
    out = np.empty((B, N, E), np.float32)
    for c in range(NCORES):
        b, half = c // 2, c % 2
        out[b, half * NQ : (half + 1) * NQ] = res.results[c]["out"]
    if _trace:
        _CACHE["last_result"] = res
    return out


# revision 12
# speedup vs baseline: 1.6455x; 1.0456x over previous
"""Multi-head attention TRN2 kernel (b=4, n=4096, e=128, h=4, d=32).

Sharding: 16 (batch, query-half) units over 8 cores; core c handles batch
c//2, query rows (c%2)*2048..+2048.  Each core computes q/k/v projections
for its batch (k,v over all 4096 keys), 4 attention heads over its 2048
query rows, and the output projection for those rows.  The host only
permutes/transposes inputs and concatenates outputs.

On-device layouts are transpose-free end to end:
  scoresT[k,q] = matmul(lhsT=kT_h, rhs=qT_h)       (K=32, head row-groups)
  expT = Exp(scoresT/sqrt(e))                       (ScalarE, the bottleneck)
  attT[hd+sum, q] = matmul(lhsT=[v_h|1], rhs=expT)  (per-head PSUM bank)
  out[q, e] = matmul(lhsT=attT_norm, rhs=W_proj)
Softmax max-subtraction is skipped (logits are ~N(0, 0.25), |logit|<6),
the value/proj biases are folded into one effective bias on the host.
"""

import os
import sys

sys.path.insert(0, "/opt/trn_rl_repo")
os.environ.setdefault("NEURON_RT_RESET_CORES", "1")

import numpy as np

E, H, D = 128, 4, 32
B, N = 4, 4096
NCORES = 8
NQ = N // 2  # per-core query rows
QB = 512  # query block
NKB = N // 128  # 32 key chunks
SCALE = float(1.0 / np.sqrt(np.float32(E)))

_CACHE = {}


def _split_multi_waits(nc):
    """This neuronxcc build accepts at most ONE sync wait per instruction;
    Tile emits up to two.  Hoist extra waits onto same-engine NoOps."""
    from concourse import mybir as mb

    for fn in nc.m.functions:
        for blk in fn.blocks:
            insts = list(blk.instructions)
            if not any(
                i.sync_info and i.sync_info.on_wait and len(i.sync_info.on_wait) > 1
                for i in insts
            ):
                continue
            new = []
            for inst in insts:
                si = inst.sync_info
                if si is not None and si.on_wait and len(si.on_wait) > 1:
                    waits = list(si.on_wait)
                    for j, w in enumerate(waits[:-1]):
                        new.append(
                            mb.InstNoOp(
                                name=f"{inst.name}-wsplit{j}",
                                engine=inst.engine,
                                ins=[],
                                outs=[],
                                sync_info=mb.SyncInfo(on_wait=[w], on_update=[]),
                            )
                        )
                    inst.sync_info = mb.SyncInfo(
                        on_wait=[waits[-1]], on_update=list(si.on_update or [])
                    )
                new.append(inst)
            blk.instructions = new


def _build(split=True):
    import concourse.bass as bass
    import concourse.tile as tile
    from concourse import mybir
    from concourse.vector_clock import ScopedClock, VectorClock

    f32 = mybir.dt.float32
    f32r = mybir.dt.float32r
    bf16 = mybir.dt.bfloat16

    class SplitDrainTileContext(tile.TileContext):
        """Final drain waits one-sem-per-instruction (walrus limit)."""

        def _drain_and_barrier(self, tick_clock, wait_clock):
            vc = tick_clock.global_clock
            n = len(vc)
            for p in range(n):
                t = vc[p]
                if t <= 0:
                    continue
                pvec = [0] * n
                pvec[p] = t
                nop_inst = self.nc.sync.nop()
                wait_clock.add_sem_waits(
                    nop_inst.ins, ScopedClock({None: VectorClock(pvec)})
                )
            self.nc.sync.drain()
            self.nc.all_engine_barrier()
            assert self.sems is not None
            popped = self.nc._tile_sem_poison_stack.pop()
            assert popped is self._sem_poison
            self.nc.clear_and_free_semaphores(list(self.sems.allocated().values()))
            self.nc.all_engine_barrier()

    nc = bass.Bass("TRN2", target_bir_lowering=False, debug=False, num_devices=NCORES)

    xT_kv = nc.dram_tensor("xT_kv", [E, N], f32, kind="ExternalInput")
    xT_q = nc.dram_tensor("xT_q", [E, NQ], f32, kind="ExternalInput")
    Wq = nc.dram_tensor("Wq", [E, E], f32, kind="ExternalInput")
    Wk = nc.dram_tensor("Wk", [E, E], f32, kind="ExternalInput")
    Wv = nc.dram_tensor("Wv", [E, E], f32, kind="ExternalInput")
    Wp = nc.dram_tensor("Wp", [E, E], f32, kind="ExternalInput")
    bq = nc.dram_tensor("bq", [E, 1], f32, kind="ExternalInput")
    bk = nc.dram_tensor("bk", [E, 1], f32, kind="ExternalInput")
    bp = nc.dram_tensor("bp", [1, E], f32, kind="ExternalInput")
    out = nc.dram_tensor("out", [NQ, E], f32, kind="ExternalOutput")

    with SplitDrainTileContext(nc) as tc:
        import contextlib

        with contextlib.ExitStack() as ctx:
            consts = ctx.enter_context(tc.tile_pool(name="consts", bufs=1))
            data = ctx.enter_context(tc.tile_pool(name="data", bufs=1))
            expool = ctx.enter_context(tc.tile_pool(name="expool", bufs=6))
            nrm = ctx.enter_context(tc.tile_pool(name="nrm", bufs=4))
            outp = ctx.enter_context(tc.tile_pool(name="outp", bufs=2))

            # ---- x loads first (longest pole for the first matmul) ----
            xq_s = data.tile([E, NQ], f32)
            xkv_s = data.tile([E, N], f32)
            for j in range(0, NQ, QB):
                nc.gpsimd.dma_start(out=xq_s[:, j : j + QB], in_=xT_q[:, j : j + QB])
            for j in range(0, N, 1024):
                nc.gpsimd.dma_start(
                    out=xkv_s[:, j : j + 1024], in_=xT_kv[:, j : j + 1024]
                )

            # ---- constants ----
            wq_s = consts.tile([E, E], f32)
            nc.gpsimd.dma_start(out=wq_s[:], in_=Wq[:])
            wk_s = consts.tile([E, E], f32)
            nc.gpsimd.dma_start(out=wk_s[:], in_=Wk[:])
            wv_s = consts.tile([E, E], f32)
            nc.gpsimd.dma_start(out=wv_s[:], in_=Wv[:])
            wp_s = consts.tile([E, E], f32)
            nc.gpsimd.dma_start(out=wp_s[:], in_=Wp[:])
            bq_s = consts.tile([E, 1], f32)
            nc.gpsimd.dma_start(out=bq_s[:], in_=bq[:])
            bk_s = consts.tile([E, 1], f32)
            nc.gpsimd.dma_start(out=bk_s[:], in_=bk[:])
            # proj bias broadcast across partitions: [1,E] -> [128,E]
            bp_s = consts.tile([E, E], f32)
            bp_bcast = bass.AP(
                tensor=bp.ap().tensor,
                offset=bp.ap().offset,
                ap=[[0, E], [1, E]],
            )
            nc.gpsimd.dma_start(out=bp_s[:], in_=bp_bcast)


            # ---- qkv projections (f32r matmuls, full-array) ----
            qT = data.tile([E, NQ], bf16)  # [ (h d), q ] with q-bias added
            kT = data.tile([E, N], bf16)  # [ (h d), k ] with k-bias added
            v1 = data.tile([E, NKB, H, D], bf16)
            ones_s = consts.tile([E, D], bf16)
            nc.vector.memset(ones_s[:], 1.0)

            pssc = ctx.enter_context(tc.tile_pool(name="pssc", bufs=3, space="PSUM"))
            psatt = ctx.enter_context(tc.tile_pool(name="psatt", bufs=2, space="PSUM"))

            def emit_qT():
                for j in range(0, NQ, QB):
                    ps = pssc.tile([E, QB], f32, tag="scps", name=f"qps{j}")
                    nc.tensor.matmul(
                        ps[:], wq_s[:], xq_s[:, j : j + QB], start=True, stop=True
                    )
                    nc.vector.tensor_scalar_add(qT[:, j : j + QB], ps[:], bq_s[:])

            def emit_kT_chunk(c):
                j = c * QB
                ps = pssc.tile([E, QB], f32, tag="scps", name=f"kps{j}")
                nc.tensor.matmul(
                    ps[:], wk_s[:], xkv_s[:, j : j + QB], start=True, stop=True
                )
                nc.vector.tensor_scalar_add(kT[:, j : j + QB], ps[:], bk_s[:])

            def emit_v_chunk(m):
                ps = pssc.tile([E, E], f32, tag="scps", name=f"vps{m}")
                nc.tensor.matmul(
                    ps[:],
                    xkv_s[:, 128 * m : 128 * m + 128],
                    wv_s[:],
                    start=True,
                    stop=True,
                )
                for h in range(H):
                    nc.vector.tensor_copy(v1[:, m, h, :], ps[:, D * h : D * h + D])

            emit_qT()
            emit_kT_chunk(0)

            # ---- attention ----
            def emit_norm(iq, att_r):
                """normalize query block iq (frees its psum accumulators)"""
                att_ps, r_ps = att_r
                rinv = nrm.tile([E, QB], f32, tag="rinv", name=f"ri{iq}")
                nc.vector.reciprocal(rinv[:], r_ps[:])
                attnT = nrm.tile([E, QB], f32, tag="attnT", name=f"attnT{iq}")
                nc.vector.tensor_mul(attnT[:], att_ps[:], rinv[:])
                return attnT

            def emit_proj(iq, attnT):
                """project + store query block iq"""
                q0 = iq * QB
                pp = pssc.tile([E, QB], f32, tag="scps", name=f"pp{iq}")
                for m in range(QB // 128):
                    nc.tensor.matmul(
                        pp[:, 128 * m : 128 * m + 128],
                        attnT[:, 128 * m : 128 * m + 128],
                        wp_s[:],
                        start=(m == 0),
                        stop=(m == QB // 128 - 1),
                        skip_group_check=True,
                    )
                ob = outp.tile([E, QB], f32, tag="ob", name=f"ob{iq}")
                bp_rep = bass.AP(
                    tensor=bp_s[:].tensor,
                    offset=bp_s[:].offset,
                    ap=[list(bp_s[:].ap[0]), [0, QB // 128], [1, E]],
                )
                ob_v = ob[:].rearrange("p (m e) -> p m e", e=E)
                pp_v = pp[:].rearrange("p (m e) -> p m e", e=E)
                nc.vector.tensor_add(ob_v, pp_v, bp_rep)
                for m in range(QB // 128):
                    nc.gpsimd.dma_start(
                        out=out[q0 + 128 * m : q0 + 128 * m + 128, :],
                        in_=ob[:, 128 * m : 128 * m + 128],
                    )

            NSC = NQ // QB * NKB * 2  # 256 half-iterations (qb, k, pair)

            def sc_tile(jj):
                return pssc.tile([E, 2 * QB], f32, tag="scps", name=f"sc{jj}")

            def emit_sc(jj, sc):
                qb, k, p = jj // (2 * NKB), (jj % (2 * NKB)) // 2, jj % 2
                q0, k0 = qb * QB, 128 * k
                for hh in range(2):
                    h = 2 * p + hh
                    nc.tensor.matmul(
                        sc[:, QB * hh : QB * hh + QB],
                        kT[D * h : D * h + D, k0 : k0 + 128],
                        qT[D * h : D * h + D, q0 : q0 + QB],
                        start=True,
                        stop=True,
                        tile_position=(D * h, 0),
                    )

            acc = {}
            pending_proj = []
            scs = {0: sc_tile(0)}
            emit_sc(0, scs[0])
            for jj in range(NSC):
                qb, k, p = jj // (2 * NKB), (jj % (2 * NKB)) // 2, jj % 2
                if qb == 0 and p == 0:
                    # stream the kv projections under the first query block
                    if k % 4 == 0 and (k // 4 + 1) < N // QB:
                        emit_kT_chunk(k // 4 + 1)
                    emit_v_chunk(k)
                if k == 0 and p == 0:
                    att_ps = psatt.tile([E, QB], f32, tag="attps", name=f"attp{qb}")
                    r_ps = psatt.tile([E, QB], f32, tag="attps", name=f"rp{qb}")
                    acc[qb] = (att_ps, r_ps)
                if jj + 1 < NSC:
                    scs[jj + 1] = sc_tile(jj + 1)
                    emit_sc(jj + 1, scs[jj + 1])
                sc = scs.pop(jj)
                ex = expool.tile([E, 2 * QB], bf16, tag="ex", name=f"ex{jj}")
                nc.scalar.activation(
                    out=ex[:],
                    in_=sc[:],
                    func=mybir.ActivationFunctionType.Exp,
                    scale=SCALE,
                )
                att_ps, r_ps = acc[qb]
                for hh in range(2):
                    h = 2 * p + hh
                    nc.tensor.matmul(
                        att_ps[D * h : D * h + D, :],
                        v1[:, k, h, :],
                        ex[:, QB * hh : QB * hh + QB],
                        start=(k == 0),
                        stop=(k == NKB - 1),
                        tile_position=(0, D * h),
                        skip_group_check=True,
                    )
                    nc.tensor.matmul(
                        r_ps[D * h : D * h + D, :],
                        ones_s[:],
                        ex[:, QB * hh : QB * hh + QB],
                        start=(k == 0),
                        stop=(k == NKB - 1),
                        tile_position=(0, D * h),
                        skip_group_check=True,
                    )
                if k == NKB - 1 and p == 1:
                    pending_proj.append((qb, emit_norm(qb, acc.pop(qb))))
                if k == 2 and p == 0 and pending_proj:
                    piq, pattnT = pending_proj.pop(0)
                    emit_proj(piq, pattnT)
            while pending_proj:
                piq, pattnT = pending_proj.pop(0)
                emit_proj(piq, pattnT)

    if split:
        _split_multi_waits(nc)
    return nc


def _prep_host(x, W_qkv, b_qkv, W_proj, b_proj):
    j = np.arange(E)
    h, d = j // D, j % D
    cq = h * (3 * D) + d * 3 + 0
    ck = cq + 1
    cv = cq + 2
    Wq = np.ascontiguousarray(W_qkv[:, cq], np.float32)
    Wk = np.ascontiguousarray(W_qkv[:, ck], np.float32)
    Wv = np.ascontiguousarray(W_qkv[:, cv], np.float32)
    bq = np.ascontiguousarray(b_qkv[cq].reshape(E, 1), np.float32)
    bk = np.ascontiguousarray(b_qkv[ck].reshape(E, 1), np.float32)
    bv = b_qkv[cv].astype(np.float32)
    bp = (bv @ W_proj + b_proj).astype(np.float32).reshape(1, E)
    Wp = np.ascontiguousarray(W_proj, np.float32)
    in_maps = []
    for c in range(NCORES):
        b, half = c // 2, c % 2
        xT_kv = np.ascontiguousarray(x[b].T, np.float32)
        xT_q = np.ascontiguousarray(x[b, half * NQ : (half + 1) * NQ].T, np.float32)
        in_maps.append(
            {
                "xT_kv": xT_kv,
                "xT_q": xT_q,
                "Wq": Wq,
                "Wk": Wk,
                "Wv": Wv,
                "Wp": Wp,
                "bq": bq,
                "bk": bk,
                "bp": bp,
            }
        )
    return in_maps


def kernel(x, W_qkv, b_qkv, W_proj, b_proj, _trace=False):
    x = np.asarray(x, np.float32)
    W_qkv = np.asarray(W_qkv, np.float32)
    b_qkv = np.asarray(b_qkv, np.float32)
    W_proj = np.asarray(W_proj, np.float32)
    b_proj = np.asarray(b_proj, np.float32)

    from concourse.bass_utils import run_bass_kernel_spmd

    if "nc" not in _CACHE:
        _CACHE["nc"] = _build()
    nc = _CACHE["nc"]

    in_maps = _prep_host(x, W_qkv, b_qkv, W_proj, b_proj)
    res = run_bass_kernel_spmd(
        nc, in_maps, core_ids=list(range(NCORES)), trace=_trace
    )
    out = np.empty((B, N, E), np.float32)
    for c in range(NCORES):
        b, half = c // 2, c % 2
        out[b, half * NQ : (half + 1) * NQ] = res.results[c]["out"]
    if _trace:
        _CACHE["last_result"] = res
    return out
